# revision 1
# baseline (speedup 1.0000x reference)
"""CASSI forward kernel for Trainium2, SPMD across 8 NeuronCores.

Computation (per batch sample b):
    y2[i, c]     = sum_l x[l, i, c-2l] * phi[i, c-2l]         (scatter-accumulate)
    out[l, i, j] = y2[i, 2l+j] * phi[i, j]                    (windowed gather)

Sharding: data-parallel over batch (B=8 -> one sample per core), phi
replicated. The pass is pure HBM traffic (x read once, out written once);
at f32 that is 59MB/core. The kernel runs fp16 I/O to halve the floor: the
host casts x/phi to fp16 (phi is binary, so the mask-multiplies introduce
no rounding), the device accumulates in PSUM f32, and the host casts the
fp16 result back to f32. Measured rel err vs the f32 reference: 5.0e-4.

Design facts this schedule is built on (all HW-measured here):
 - Loads are read-latency bound: 253 GB/s on one HWDGE queue, 310 on two;
   stores reach ~345 on one. Mixing directions caps ~280 combined, so the
   schedule is two pure phases: load everything (accumulators stay
   on-chip), then store everything, both alternating sync/scalar queues.
 - DVE and GpSimd serialize on an exclusive shared SBUF port pair (every
   2-source DVE op locks it; Pool is also ~3.7x slower per element), so
   GpSimd is left idle. PE and ACT have dedicated ports and run parallel.
 - The scatter-accumulate rides PE as identity matmuls into PSUM f32
   (pieces split at 512-f32 bank boundaries; dispersion tails pre-armed
   with zero matmuls, group closed with stop=True zero matmuls). ACT exits
   PSUM to SBUF as fp16 (~1us; y2 is only 128 x 2*566), which keeps the
   windowed stage-2 multiplies on DVE in fp16 2x mode (all operands
   2-byte, unit-stride innermost runs). DVE does only the mask-multiplies
   (~31us) and stage-2 (~31us), under the ~90us phased-DMA floor.
 - Row-pair layout (partition p holds rows r0+2p, r0+2p+1) makes every
   per-band transfer one jump-free 256KB stream of 2KB runs. (The "pequad"
   body packs 4 rows -> 4KB runs / 353 GB/s, splitting y2 across two
   PSUM tiles with rows padded to 1024 f32; it measures no faster because
   PE's 224 self-loading identity matmuls then pace the load phase.)
 - Stage-2 is emitted eagerly per block into a deep SBUF ot pool so the
   store phase starts the moment the last load lands.

Measured via For_i(1001) marginal timing, device-resident inputs: ~92us
per pass (stable at 92+-1 across ~10 quiet-window runs; the rebal=True
queue-byte-balance variant measured 86 then 100 in two rounds - noise, as
theory predicts only ~0.8us - so it stays off) vs the 270us fp32
baseline (2.9x) and a ~104us mixed-direction
DMA floor; loads 47.3 + stores 42.5 + ~1.9 ramp = the full makespan, i.e.
the schedule sits at its phased-DMA floor. Raising either phase bandwidth
is blocked at this API level: fp16 PE streaming is capped at 1 col/cycle
(fp8's 2x DoubleRow mode fails the precision gate), InstMatmult cannot be
emitted weights-resident (2-input schema; bacc owns the ldweights split),
and a third DMA queue via SWDGE loses more to GpSimd descriptor-gen
lockout (DVE TT ops hold the shared SBUF port pair) than it gains.
"""

import sys

if "/opt/trn_rl_repo" not in sys.path:
    sys.path.insert(0, "/opt/trn_rl_repo")

import numpy as np

import concourse.bass as bass
import concourse.bacc as bacc
import concourse.mybir as mybir
import concourse.tile as tile
from concourse.bass_utils import run_bass_kernel_spmd

B = 8
L, M, N, S = 28, 512, 512, 2
NOUT = N + S * (L - 1)  # 566
P = 128

NP_DT = np.float16

RG = 2               # rows per partition (row-pair layout, 2KB runs)
RBLK = M // (P * RG)  # 2 row-blocks of 256 rows
RW = RG * N          # per-partition elements per band
GB = 4               # bands per load/mult tile
GS = 4               # bands per stage-2 group

_cached = {}

# Production configuration used by kernel(); _build_nc(loop) with no cfg
# also resolves to this so the timing harness measures the same schedule.
PROD_CFG = {"body": "pe", "eager": True, "obufs": 14, "xbufs": 6}


def _body16(nc, tc, x_d, phi_d, out_d, cfg=None):
    cfg = dict(cfg or {})
    loads = cfg.get("loads", True)
    mult = cfg.get("mult", "pool")  # 'pool' | 'dve' | 'split' | None
    mult_f = cfg.get("mult_f", 5 / 14)  # DVE share when mult == 'split'
    adds = cfg.get("adds", True)
    stage2 = cfg.get("stage2", True)
    stores = cfg.get("stores", True)
    split_tail = cfg.get("split_tail", True)
    loads_2q = cfg.get("loads_2q", False)
    xbufs = cfg.get("xbufs", 6)
    gb = cfg.get("gb", GB)
    gs = cfg.get("gs", GS)
    rg = cfg.get("rg", RG)
    rblk = M // (P * rg)
    rw = rg * N
    ngrp = (L + gb - 1) // gb
    f16 = mybir.dt.float16
    with (
        tc.tile_pool(name="phip", bufs=1) as phi_pool,
        tc.tile_pool(name="y2p", bufs=2) as y2_pool,
        tc.tile_pool(name="xp", bufs=xbufs) as x_pool,
        tc.tile_pool(name="op", bufs=3) as o_pool,
    ):
        # phi -> SBUF once, both blocks' row-pair layouts side by side.
        # Rides the (store-only) Activation queue so x loads start at t=0.
        phi_sb = phi_pool.tile([P, rblk * rw], f16)
        nc.scalar.dma_start(
            phi_sb[:, :].rearrange("p (b q) -> p b q", q=rw),
            phi_d.rearrange("(b p r) n -> p b (r n)", b=rblk, r=rg),
        )

        for b in range(rblk):
            r0 = b * P * rg
            last = b == rblk - 1
            phi_blk = phi_sb[:, b * rw : (b + 1) * rw]

            y2 = y2_pool.tile([P, rg * NOUT], f16)
            if adds is True:
                # band 0's accumulate is a direct copy, so only the
                # dispersion tails [N, NOUT) of each row need zeroing
                tail = bass.AP(
                    y2.tensor,
                    y2[:, N : N + 1].offset,
                    [list(y2[:, :].ap[0]), [NOUT, rg], [1, NOUT - N]],
                )
                nc.vector.memset(tail, 0.0)
            else:
                nc.vector.memset(y2[:, :], 0.0)

            for l0 in range(0, L, gb):
                xt = x_pool.tile([P, gb * rw], f16)
                # Per-band transfers: each is a single 256KB jump-free
                # stream (2KB per partition, contiguous across partitions).
                if loads:
                    for j in range(gb):
                        ld_eng = (
                            nc.scalar
                            if (loads_2q and (l0 + j) % 2 == 1)
                            else nc.sync
                        )
                        ld_eng.dma_start(
                            xt[:, j * rw : (j + 1) * rw],
                            x_d[l0 + j, r0 : r0 + P * rg, :].rearrange(
                                "(p r) n -> p (r n)", r=rg
                            ),
                        )
                else:
                    # sliver write so the tile is allocated; disjoint from
                    # the ranges compute reads, so nothing gates on it
                    nc.vector.memset(xt[0:1, 0:2], 0.0)
                x4 = bass.AP(
                    xt.tensor,
                    xt[:, :].offset,
                    [list(xt[:, :].ap[0]), [rw, gb], [N, rg], [1, N]],
                )
                phi_m = bass.AP(
                    phi_blk.tensor,
                    phi_blk.offset,
                    [list(phi_blk.ap[0]), [0, gb], [N, rg], [1, N]],
                )
                if mult == "split":
                    # GPSIMD's software Multiply runs at 0.42 of roofline
                    # (~114us for the full mask-multiply); DVE in fp16 2x
                    # mode has headroom under the DMA floor. Hand mult_f of
                    # the band groups to DVE (Bresenham spacing) so both
                    # engines land at ~71us.
                    gi = b * ngrp + l0 // gb
                    dve_turn = int((gi + 1) * mult_f) > int(gi * mult_f)
                    eng = nc.vector if dve_turn else nc.gpsimd
                    eng.tensor_tensor(x4, x4, phi_m, mybir.AluOpType.mult)
                elif mult == "pool":
                    nc.gpsimd.tensor_tensor(
                        x4, x4, phi_m, mybir.AluOpType.mult
                    )
                elif mult == "dve":
                    nc.vector.tensor_tensor(
                        x4, x4, phi_m, mybir.AluOpType.mult
                    )
                if not adds:
                    continue
                for j in range(gb):
                    l = l0 + j
                    if adds == "contig":
                        # cost probe: same elem count, flat unit-stride APs
                        dst = y2[:, 0:rw]
                        src = xt[:, j * rw : (j + 1) * rw]
                        nc.vector.tensor_tensor(
                            dst, dst, src, mybir.AluOpType.add
                        )
                        continue
                    dst = bass.AP(
                        y2.tensor,
                        y2[:, S * l : S * l + 1].offset,
                        [list(y2[:, :].ap[0]), [NOUT, rg], [1, N]],
                    )
                    src = bass.AP(
                        xt.tensor,
                        xt[:, j * rw : j * rw + 1].offset,
                        [list(xt[:, :].ap[0]), [N, rg], [1, N]],
                    )
                    if l == 0:
                        nc.vector.tensor_copy(dst, src)
                    else:
                        nc.vector.tensor_tensor(
                            dst, dst, src, mybir.AluOpType.add
                        )

            for l0 in range(0, L, gs):
                g = min(gs, L - l0)
                ot = o_pool.tile([P, gs * rw], f16)
                o4 = bass.AP(
                    ot.tensor,
                    ot[:, :].offset,
                    [list(ot[:, :].ap[0]), [rw, g], [N, rg], [1, N]],
                )
                win = bass.AP(
                    y2.tensor,
                    y2[:, S * l0 : S * l0 + 1].offset,
                    [list(y2[:, :].ap[0]), [S, g], [NOUT, rg], [1, N]],
                )
                phi4 = bass.AP(
                    phi_blk.tensor,
                    phi_blk.offset,
                    [list(phi_blk.ap[0]), [0, g], [N, rg], [1, N]],
                )
                if stage2 == "contig":
                    # cost probe: same elem count, flat unit-stride APs
                    for j in range(g):
                        nc.vector.tensor_tensor(
                            ot[:, j * rw : (j + 1) * rw],
                            y2[:, 0:rw],
                            phi_blk,
                            mybir.AluOpType.mult,
                        )
                elif stage2:
                    nc.vector.tensor_tensor(
                        o4, win, phi4, mybir.AluOpType.mult
                    )
                else:
                    nc.vector.memset(ot[0:1, 0:2], 0.0)
                if not stores:
                    continue
                for j in range(g):
                    l = l0 + j
                    # Tail drain: the last block's stores alternate between
                    # both HWDGE queues (the load queue is idle by then).
                    st_eng = (
                        nc.sync
                        if (split_tail and last and l % 2 == 0)
                        else nc.scalar
                    )
                    st_eng.dma_start(
                        out_d[l, r0 : r0 + P * rg, :].rearrange(
                            "(p r) n -> p (r n)", r=rg
                        ),
                        ot[:, j * rw : (j + 1) * rw],
                    )


def _body_2phase(nc, tc, x_d, phi_d, out_d, cfg=None):
    """Two-phase schedule: the HBM streams run ~25% faster when the
    directions don't mix (loads 2q: 310 GB/s, stores 1-2q: 345 GB/s, vs
    ~280 GB/s combined when concurrent). Phase L: all loads alternating
    across both HWDGE queues, with mult+adds chasing on DVE/Pool into the
    two SBUF-resident y2 accumulators. Phase S: windowed stage-2 + all
    stores, also alternating across both queues."""
    cfg = dict(cfg or {})
    mult_f = cfg.get("mult_f", 0.4)  # fraction of mult groups on Pool
    s2_pool = cfg.get("s2_pool", 0)  # stage2 groups (from block 0) on Pool
    xbufs = cfg.get("xbufs", 8)
    gb = cfg.get("gb", GB)
    gs = cfg.get("gs", GS)
    rg = cfg.get("rg", RG)
    rblk = M // (P * rg)
    rw = rg * N
    ngrp = (L + gb - 1) // gb
    f16 = mybir.dt.float16
    with (
        tc.tile_pool(name="phip", bufs=1) as phi_pool,
        tc.tile_pool(name="y2p", bufs=rblk) as y2_pool,
        tc.tile_pool(name="xp", bufs=xbufs) as x_pool,
        tc.tile_pool(name="op", bufs=3) as o_pool,
    ):
        phi_sb = phi_pool.tile([P, rblk * rw], f16)
        nc.scalar.dma_start(
            phi_sb[:, :].rearrange("p (b q) -> p b q", q=rw),
            phi_d.rearrange("(b p r) n -> p b (r n)", b=rblk, r=rg),
        )

        y2s = []
        for b in range(rblk):
            r0 = b * P * rg
            phi_blk = phi_sb[:, b * rw : (b + 1) * rw]
            y2 = y2_pool.tile([P, rg * NOUT], f16)
            y2s.append((y2, phi_blk, r0))
            tail = bass.AP(
                y2.tensor,
                y2[:, N : N + 1].offset,
                [list(y2[:, :].ap[0]), [NOUT, rg], [1, NOUT - N]],
            )
            nc.vector.memset(tail, 0.0)

            for l0 in range(0, L, gb):
                xt = x_pool.tile([P, gb * rw], f16)
                for j in range(gb):
                    ld_eng = nc.scalar if (l0 + j) % 2 else nc.sync
                    ld_eng.dma_start(
                        xt[:, j * rw : (j + 1) * rw],
                        x_d[l0 + j, r0 : r0 + P * rg, :].rearrange(
                            "(p r) n -> p (r n)", r=rg
                        ),
                    )
                x4 = bass.AP(
                    xt.tensor,
                    xt[:, :].offset,
                    [list(xt[:, :].ap[0]), [rw, gb], [N, rg], [1, N]],
                )
                phi_m = bass.AP(
                    phi_blk.tensor,
                    phi_blk.offset,
                    [list(phi_blk.ap[0]), [0, gb], [N, rg], [1, N]],
                )
                gi = b * ngrp + l0 // gb
                pool_turn = int((gi + 1) * mult_f) > int(gi * mult_f)
                eng = nc.gpsimd if pool_turn else nc.vector
                eng.tensor_tensor(x4, x4, phi_m, mybir.AluOpType.mult)
                for j in range(gb):
                    l = l0 + j
                    dst = bass.AP(
                        y2.tensor,
                        y2[:, S * l : S * l + 1].offset,
                        [list(y2[:, :].ap[0]), [NOUT, rg], [1, N]],
                    )
                    src = bass.AP(
                        xt.tensor,
                        xt[:, j * rw : j * rw + 1].offset,
                        [list(xt[:, :].ap[0]), [N, rg], [1, N]],
                    )
                    if l == 0:
                        nc.vector.tensor_copy(dst, src)
                    else:
                        nc.vector.tensor_tensor(
                            dst, dst, src, mybir.AluOpType.add
                        )

        for b, (y2, phi_blk, r0) in enumerate(y2s):
            for si, l0 in enumerate(range(0, L, gs)):
                g = min(gs, L - l0)
                ot = o_pool.tile([P, gs * rw], f16)
                o4 = bass.AP(
                    ot.tensor,
                    ot[:, :].offset,
                    [list(ot[:, :].ap[0]), [rw, g], [N, rg], [1, N]],
                )
                win = bass.AP(
                    y2.tensor,
                    y2[:, S * l0 : S * l0 + 1].offset,
                    [list(y2[:, :].ap[0]), [S, g], [NOUT, rg], [1, N]],
                )
                phi4 = bass.AP(
                    phi_blk.tensor,
                    phi_blk.offset,
                    [list(phi_blk.ap[0]), [0, g], [N, rg], [1, N]],
                )
                s2_eng = (
                    nc.gpsimd if (b == 0 and si < s2_pool) else nc.vector
                )
                s2_eng.tensor_tensor(o4, win, phi4, mybir.AluOpType.mult)
                for j in range(g):
                    l = l0 + j
                    st_eng = nc.sync if l % 2 == 0 else nc.scalar
                    st_eng.dma_start(
                        out_d[l, r0 : r0 + P * rg, :].rearrange(
                            "(p r) n -> p (r n)", r=rg
                        ),
                        ot[:, j * rw : (j + 1) * rw],
                    )


def _bank_pieces(a, b, bank=512):
    """Split [a, b) at PSUM bank boundaries (512 f32 elems per bank)."""
    out = []
    while a < b:
        nxt = min(b, (a // bank + 1) * bank)
        out.append((a, nxt))
        a = nxt
    return out


def _body_pe16(nc, tc, x_d, phi_d, eye_d, out_d, cfg=None):
    """PE-scatter two-phase schedule.

    DVE and GpSimd serialize on an exclusive shared SBUF port pair (every
    2-source DVE op locks it), so GpSimd cannot offload DVE. PE and ACT
    have their own ports and run truly in parallel. The scatter-accumulate
    therefore rides PE: per band, identity matmuls accumulate the masked
    slab into a PSUM-resident f32 y2 (split at bank boundaries); ACT then
    copies y2 to SBUF as fp16 (~1us - y2 is tiny), which keeps stage-2 on
    DVE in fp16 2x mode. DVE does only the mask-multiplies (~31us) and
    stage-2 windows (~31us), under the phased-DMA floor (~90us).

    Phase L: all loads alternate both HWDGE queues (310-353 GB/s pure-read
    vs ~280 mixed). Phase S: stage-2 + all stores, alternating queues.
    """
    cfg = dict(cfg or {})
    xbufs = cfg.get("xbufs", 8)
    gb = cfg.get("gb", GB)
    gs = cfg.get("gs", GS)
    rg = cfg.get("rg", RG)
    eager = cfg.get("eager", False)
    obufs = cfg.get("obufs", 8 if eager else 3)
    ldq = cfg.get("ldq", ["sync", "scalar"])
    phiq = cfg.get("phiq", "scalar")
    rebal = cfg.get("rebal", False)
    early_st = cfg.get("early_st", False) and eager
    rblk = M // (P * rg)
    rw = rg * N
    f16 = mybir.dt.float16
    f32 = mybir.dt.float32
    with (
        tc.tile_pool(name="phip", bufs=1) as phi_pool,
        tc.tile_pool(name="ypsum", bufs=rblk, space="PSUM") as y_pool,
        tc.tile_pool(name="ysb", bufs=rblk) as ysb_pool,
        tc.tile_pool(name="xp", bufs=xbufs) as x_pool,
        tc.tile_pool(name="op", bufs=obufs) as o_pool,
    ):
        phi_sb = phi_pool.tile([P, rblk * rw], f16)
        getattr(nc, phiq).dma_start(
            phi_sb[:, :].rearrange("p (b q) -> p b q", q=rw),
            phi_d.rearrange("(b p r) n -> p b (r n)", b=rblk, r=rg),
        )
        eye_sb = phi_pool.tile([P, P], f16)
        nc.sync.dma_start(eye_sb[:, :], eye_d)
        zeros_sb = phi_pool.tile([P, 512], f16)
        nc.vector.memset(zeros_sb[:, :], 0.0)

        blocks = []
        ots = {}

        def _emit_stage2(b):
            y2s, phi_blk, r0 = blocks[b]
            ots[b] = []
            for l0 in range(0, L, gs):
                g = min(gs, L - l0)
                ot = o_pool.tile([P, gs * rw], f16)
                ots[b].append((ot, l0, g))
                o4 = bass.AP(
                    ot.tensor,
                    ot[:, :].offset,
                    [list(ot[:, :].ap[0]), [rw, g], [N, rg], [1, N]],
                )
                win = bass.AP(
                    y2s.tensor,
                    y2s[:, S * l0 : S * l0 + 1].offset,
                    [list(y2s[:, :].ap[0]), [S, g], [NOUT, rg], [1, N]],
                )
                phi4 = bass.AP(
                    phi_blk.tensor,
                    phi_blk.offset,
                    [list(phi_blk.ap[0]), [0, g], [N, rg], [1, N]],
                )
                nc.vector.tensor_tensor(o4, win, phi4, mybir.AluOpType.mult)

        def _emit_stores(b):
            _, _, r0 = blocks[b]
            for si, (ot, l0, g) in enumerate(ots[b]):
                if early_st and b == 0 and si == 0:
                    continue  # already emitted ahead of the last loads
                for j in range(g):
                    l = l0 + j
                    st_eng = nc.sync if l % 2 == 0 else nc.scalar
                    st_eng.dma_start(
                        out_d[l, r0 : r0 + P * rg, :].rearrange(
                            "(p r) n -> p (r n)", r=rg
                        ),
                        ot[:, j * rw : (j + 1) * rw],
                    )

        for b in range(rblk):
            r0 = b * P * rg
            phi_blk = phi_sb[:, b * rw : (b + 1) * rw]
            y2p = y_pool.tile([P, rg * NOUT], f32)
            y2s = ysb_pool.tile([P, rg * NOUT], f16)
            blocks.append((y2s, phi_blk, r0))
            # arm the dispersion tails [N, NOUT) of each row: first writer
            # of a PSUM region must carry start=True
            for r in range(rg):
                for a, e in _bank_pieces(r * NOUT + N, (r + 1) * NOUT):
                    nc.tensor.matmul(
                        y2p[:, a:e],
                        eye_sb[:, :],
                        zeros_sb[:, 0 : e - a],
                        start=True,
                        stop=False,
                    )

            for l0 in range(0, L, gb):
                if early_st and b == rblk - 1 and l0 == L - gb:
                    # slot block 0's first stage-2 group's stores ahead of
                    # the final load group in the engine streams: they fill
                    # any boundary bubble (their ot has long been ready)
                    ot0, s_l0, s_g = ots[0][0]
                    for j in range(s_g):
                        l = s_l0 + j
                        st_eng = nc.sync if l % 2 == 0 else nc.scalar
                        st_eng.dma_start(
                            out_d[
                                l, 0 : P * rg, :
                            ].rearrange("(p r) n -> p (r n)", r=rg),
                            ot0[:, j * rw : (j + 1) * rw],
                        )
                xt = x_pool.tile([P, gb * rw], f16)
                for j in range(gb):
                    ld_eng = getattr(nc, ldq[(l0 + j) % len(ldq)])
                    if rebal and b == rblk - 1 and l0 + j == L - 1:
                        # phi (0.5MB) rides the scalar queue, so hand its
                        # last x-band to sync: 7.28 vs 7.25MB per queue
                        # instead of 7.37 vs 7.87 - the phase ends with the
                        # slower queue
                        ld_eng = nc.sync
                    ld_eng.dma_start(
                        xt[:, j * rw : (j + 1) * rw],
                        x_d[l0 + j, r0 : r0 + P * rg, :].rearrange(
                            "(p r) n -> p (r n)", r=rg
                        ),
                    )
                x4 = bass.AP(
                    xt.tensor,
                    xt[:, :].offset,
                    [list(xt[:, :].ap[0]), [rw, gb], [N, rg], [1, N]],
                )
                phi_m = bass.AP(
                    phi_blk.tensor,
                    phi_blk.offset,
                    [list(phi_blk.ap[0]), [0, gb], [N, rg], [1, N]],
                )
                nc.vector.tensor_tensor(x4, x4, phi_m, mybir.AluOpType.mult)
                for j in range(gb):
                    l = l0 + j
                    for r in range(rg):
                        c0 = r * NOUT + S * l
                        s0 = j * rw + r * N
                        for a, e in _bank_pieces(c0, c0 + N):
                            nc.tensor.matmul(
                                y2p[:, a:e],
                                eye_sb[:, :],
                                xt[:, s0 + a - c0 : s0 + e - c0],
                                start=(l == 0),
                                stop=False,
                            )

            # close the accumulation group over the whole tile, then exit
            # PSUM through ACT (own ports; ~1us) as fp16 for 2x stage-2
            for a, e in _bank_pieces(0, rg * NOUT):
                nc.tensor.matmul(
                    y2p[:, a:e],
                    eye_sb[:, :],
                    zeros_sb[:, 0 : e - a],
                    start=False,
                    stop=True,
                )
            nc.scalar.activation(
                y2s[:, :], y2p[:, :], mybir.ActivationFunctionType.Copy
            )
            if eager:
                # emit stage-2 now (deep ot pool buffers the whole block)
                # so the store phase starts with zero stage-2 lag
                _emit_stage2(b)

        for b in range(rblk):
            if not eager:
                _emit_stage2(b)
            _emit_stores(b)


def _mm_noweights(nc, out, rhs, start, stop):
    """Non-self-loading InstMatmult: uses the PE weights left by the last
    standalone ldweights (bass's matmul() always re-streams the stationary
    operand - ~128 cycles per instruction - which paces PE when every
    matmul shares the same identity)."""
    eng = nc.tensor
    ifmap_ap = eng.lower_ap(rhs.opt({0}), opt=False)
    out_ap = eng.lower_ap(out)
    return eng.add_instruction(
        mybir.InstMatmult(
            name=eng.bass.get_next_instruction_name(),
            replication_resolution=0,
            replication_shift_amnt=0,
            replication_num_rows=0,
            start_tensor_calc=start,
            stop_tensor_calc=stop,
            ins=[ifmap_ap],
            outs=[out_ap],
            perf_mode=None,
            is_transpose=None,
            ifmap_quant_offset=None,
            weights_quant_offset=None,
            bass_skip_group_check=False,
            tile_position=(0, 0),
            tile_size=(128, 128),
        )
    )


def _body_pe_quad(nc, tc, x_d, phi_d, eye_d, out_d, cfg=None):
    """PE-scatter two-phase with quad-row load packing.

    Same engine assignment as _body_pe16, but partition p loads rows
    4p..4p+3 in one 4KB contiguous run per band (353 GB/s measured, vs 310
    for 2KB row-pairs). The accumulator is split across two PSUM tiles
    (tile t holds rows 4p+2t, 4p+2t+1) with the row stride PADDED to 1024
    f32 (one tile = 4 banks, the pair = all 8): both rows then share
    bank-crossing geometry, so each band's two rows accumulate in a single
    strided matmul (2 bank-pieces per band per tile) - half the PE
    instructions of the unpadded layout, keeping PE (~33us) under the 4KB
    load stream (~42us).
    """
    cfg = dict(cfg or {})
    xbufs = cfg.get("xbufs", 6)
    gb = cfg.get("gb", 2)
    gs = cfg.get("gs", GS)
    obufs = cfg.get("obufs", 14)
    ldw = cfg.get("ldw", False)
    dr = cfg.get("dr", False)
    dvb = cfg.get("dvb", 0)  # trailing bands accumulated on DVE, not PE
    rg, half = 4, 2
    rw = rg * N
    hw_ = half * N  # per-tile elems per band
    PAD = 1024      # padded f32 row stride inside a PSUM tile
    f16 = mybir.dt.float16
    f32 = mybir.dt.float32
    with (
        tc.tile_pool(name="phip", bufs=1) as phi_pool,
        tc.tile_pool(name="ypsum", bufs=1, space="PSUM") as y_pool,
        tc.tile_pool(name="ysb", bufs=1) as ysb_pool,
        tc.tile_pool(name="xp", bufs=xbufs) as x_pool,
        tc.tile_pool(name="op", bufs=obufs) as o_pool,
    ):
        phi_sb = phi_pool.tile([P, rw], f16)
        nc.scalar.dma_start(
            phi_sb[:, :], phi_d.rearrange("(p r) n -> p (r n)", r=rg)
        )
        eye_sb = phi_pool.tile([P, P], f16)
        nc.sync.dma_start(eye_sb[:, :], eye_d)
        zeros_sb = phi_pool.tile([P, 512], f16)
        nc.vector.memset(zeros_sb[:, :], 0.0)

        if ldw:
            nc.tensor.ldweights(eye_sb[:, :])

        pm = mybir.MatmulPerfMode.DoubleRow if dr else None

        def mm(out, rhs, start, stop):
            if ldw:
                _mm_noweights(nc, out, rhs, start, stop)
            else:
                nc.tensor.matmul(
                    out, eye_sb[:, :], rhs, start=start, stop=stop,
                    perf_mode=pm,
                )

        y2ps = [
            y_pool.tile([P, half * PAD], f32, name=f"y2p{t}")
            for t in range(2)
        ]
        y2ss = [
            ysb_pool.tile([P, half * NOUT], f16, name=f"y2s{t}")
            for t in range(2)
        ]
        y2ds = [
            ysb_pool.tile([P, half * NOUT], f16, name=f"y2d{t}")
            for t in range(2)
        ] if dvb else []
        for y2d in y2ds:
            nc.vector.memset(y2d[:, :], 0.0)

        def rows2(tile_ap, off, w, stride=PAD):
            """Both rows of a tile at [off, off+w) per row."""
            return bass.AP(
                tile_ap.tensor,
                tile_ap[:, off : off + 1].offset,
                [list(tile_ap[:, :].ap[0]), [stride, half], [1, w]],
            )

        for y2p in y2ps:
            # arm tail+pad [512, 1024) of each row (bank-aligned, virgin)
            for r in range(half):
                mm(
                    y2p[:, r * PAD + 512 : r * PAD + 1024],
                    zeros_sb[:, :],
                    True,
                    False,
                )

        for l0 in range(0, L, gb):
            xt = x_pool.tile([P, gb * rw], f16)
            for j in range(gb):
                ld_eng = nc.scalar if (l0 + j) % 2 else nc.sync
                ld_eng.dma_start(
                    xt[:, j * rw : (j + 1) * rw],
                    x_d[l0 + j, :, :].rearrange("(p r) n -> p (r n)", r=rg),
                )
            x4 = bass.AP(
                xt.tensor,
                xt[:, :].offset,
                [list(xt[:, :].ap[0]), [rw, gb], [N, rg], [1, N]],
            )
            phi_m = bass.AP(
                phi_sb.tensor,
                phi_sb[:, :].offset,
                [list(phi_sb[:, :].ap[0]), [0, gb], [N, rg], [1, N]],
            )
            nc.vector.tensor_tensor(x4, x4, phi_m, mybir.AluOpType.mult)
            for j in range(gb):
                l = l0 + j
                if l >= L - dvb:
                    # DVE has phase-L slack under the 4KB load stream;
                    # trailing bands accumulate in fp16 SBUF instead of PE
                    for t in range(2):
                        y2d = y2ds[t]
                        dst = bass.AP(
                            y2d.tensor,
                            y2d[:, S * l : S * l + 1].offset,
                            [list(y2d[:, :].ap[0]), [NOUT, half], [1, N]],
                        )
                        srcp = bass.AP(
                            xt.tensor,
                            xt[:, j * rw + t * hw_ : j * rw + t * hw_ + 1].offset,
                            [list(xt[:, :].ap[0]), [N, half], [1, N]],
                        )
                        nc.vector.tensor_tensor(
                            dst, dst, srcp, mybir.AluOpType.add
                        )
                    continue
                for t in range(2):
                    y2p = y2ps[t]
                    c0 = S * l
                    for r in range(half):
                        s0 = j * rw + (t * half + r) * N
                        for a, e in _bank_pieces(c0, c0 + N):
                            mm(
                                y2p[:, r * PAD + a : r * PAD + e],
                                xt[:, s0 + a - c0 : s0 + e - c0],
                                l == 0,
                                False,
                            )

        ots = {}
        for t in range(2):
            y2p, y2s = y2ps[t], y2ss[t]
            for r in range(half):
                for a in (0, 512):
                    mm(
                        y2p[:, r * PAD + a : r * PAD + a + 512],
                        zeros_sb[:, :],
                        False,
                        True,
                    )
            conv_in = rows2(y2p, 0, NOUT)
            conv_out = bass.AP(
                y2s.tensor,
                y2s[:, 0:1].offset,
                [list(y2s[:, :].ap[0]), [NOUT, half], [1, NOUT]],
            )
            nc.scalar.activation(
                conv_out, conv_in, mybir.ActivationFunctionType.Copy
            )
            if dvb:
                nc.vector.tensor_tensor(
                    y2s[:, :], y2s[:, :], y2ds[t][:, :],
                    mybir.AluOpType.add,
                )
            phi_t = phi_sb[:, t * hw_ : (t + 1) * hw_]
            ots[t] = []
            for l0 in range(0, L, gs):
                g = min(gs, L - l0)
                ot = o_pool.tile([P, gs * hw_], f16)
                ots[t].append((ot, l0, g))
                o4 = bass.AP(
                    ot.tensor,
                    ot[:, :].offset,
                    [list(ot[:, :].ap[0]), [hw_, g], [N, half], [1, N]],
                )
                win = bass.AP(
                    y2s.tensor,
                    y2s[:, S * l0 : S * l0 + 1].offset,
                    [list(y2s[:, :].ap[0]), [S, g], [NOUT, half], [1, N]],
                )
                phi4 = bass.AP(
                    phi_t.tensor,
                    phi_t.offset,
                    [list(phi_t.ap[0]), [0, g], [N, half], [1, N]],
                )
                nc.vector.tensor_tensor(o4, win, phi4, mybir.AluOpType.mult)

        for t in range(2):
            for ot, l0, g in ots[t]:
                for j in range(g):
                    l = l0 + j
                    st_eng = nc.sync if l % 2 == 0 else nc.scalar
                    dst = out_d[l, :, :].rearrange("(p r) n -> p r n", r=rg)[
                        :, t * half : (t + 1) * half, :
                    ]
                    st_eng.dma_start(
                        dst,
                        ot[:, j * hw_ : (j + 1) * hw_].rearrange(
                            "p (r n) -> p r n", n=N
                        ),
                    )


def _build_nc(loop: int = 1, cfg=None):
    if cfg is None:
        cfg = PROD_CFG
    body = cfg.get("body", "body16")
    nc = bacc.Bacc("TRN2", target_bir_lowering=False, debug=False)
    f16 = mybir.dt.float16
    x_d = nc.dram_tensor("x", [L, M, N], f16, kind="ExternalInput").ap()
    phi_d = nc.dram_tensor("phi", [M, N], f16, kind="ExternalInput").ap()
    eye_d = (
        nc.dram_tensor("eye", [P, P], f16, kind="ExternalInput").ap()
        if body in ("pe", "pequad")
        else None
    )
    out_d = nc.dram_tensor("out", [L, M, N], f16, kind="ExternalOutput").ap()

    def emit():
        if body == "pequad":
            _body_pe_quad(nc, tc, x_d, phi_d, eye_d, out_d, cfg)
        elif body == "pe":
            _body_pe16(nc, tc, x_d, phi_d, eye_d, out_d, cfg)
        elif body == "2phase":
            _body_2phase(nc, tc, x_d, phi_d, out_d, cfg)
        else:
            _body16(nc, tc, x_d, phi_d, out_d, cfg)

    with tile.TileContext(nc) as tc:
        if loop == 1:
            emit()
        elif loop < 0:
            with tc.For_i(0, -loop, 1):
                emit()
        else:
            # static unroll: no back-edge barriers, iterations pipeline
            for _ in range(loop):
                emit()

    nc.compile()
    return nc


def _get_nc():
    if "nc" not in _cached:
        _cached["nc"] = _build_nc()
    return _cached["nc"]


def harness_inputs(nc, batched=True):
    """Per-input global arrays (concat over the 8 cores, axis 0) keyed and
    ordered as the compiled module's ExternalInputs. Used by the timing
    harness; kernel() builds the same per-core maps itself."""
    import concourse.mybir as _mybir

    rng = np.random.default_rng(0)
    per_core = {
        "x": lambda: rng.standard_normal((L, M, N), dtype=np.float32).astype(
            NP_DT
        ),
        "phi": lambda: (rng.standard_normal((M, N)) > 0).astype(NP_DT),
        "eye": lambda: np.eye(P, dtype=NP_DT),
    }
    skip = (
        {nc.partition_id_tensor.name} if nc.partition_id_tensor else set()
    )
    names = []
    for alloc in nc.m.functions[0].allocations:
        if (
            isinstance(alloc, _mybir.MemoryLocationSet)
            and alloc.kind == "ExternalInput"
            and alloc.memorylocations[0].name not in skip
        ):
            names.append(alloc.memorylocations[0].name)
    out = []
    for name in names:
        a = per_core[name]()
        out.append(np.concatenate([a] * B, axis=0) if batched else a)
    return names, out


def kernel(x: np.ndarray, phi: np.ndarray) -> np.ndarray:
    assert x.shape == (B, L, M, N) and phi.shape == (M, N)
    nc = _get_nc()
    x16 = np.ascontiguousarray(x, dtype=np.float32).astype(np.float16)
    phi16 = np.ascontiguousarray(phi, dtype=np.float32).astype(np.float16)
    base = {"phi": phi16}
    if PROD_CFG.get("body") in ("pe", "pequad"):
        base["eye"] = np.eye(P, dtype=np.float16)
    in_maps = [dict(base, x=x16[i]) for i in range(B)]
    res = run_bass_kernel_spmd(nc, in_maps, list(range(B)))
    return np.stack(
        [r["out"].astype(np.float32) for r in res.results], axis=0
    )


if __name__ == "__main__":
    x = np.random.randn(B, L, M, N).astype(np.float32)
    phi = (np.random.randn(M, N) > 0).astype(np.float32)
    out = kernel(x, phi)
    print("out", out.shape, out.dtype)

